# revision 2
# baseline (speedup 1.0000x reference)
"""Self-contained Bass/Tile TRN2 kernel v2: 1-layer LSTM encoder, T=20,
batch 65536, hidden 64, data-parallel over batch across 8 NeuronCores.

Feature-major ("orientation B") design — h-state never leaves feature-major
layout, so there is no per-step transpose at all:

  - State tile S[67, 8192] bf16 = [h (rows 0:64); const-1 (row 64);
    x_t (rows 65:67)].  Embedding folded into the input projection on host
    (W_x = W_ih@W_emb, bias = b_ih+b_hh+W_ih@b_emb).
  - Gates per 512-batch chunk via ONE K=67 matmul per gate pair into PSUM:
    pair A = {i,f} (native ACT Sigmoid), pair B = {g,o} (ACT Tanh; o-gate
    weights pre-scaled 0.5 so sigma_o = (tanh+1)/2 via a DVE 4x fixup).
  - The cell state c is packed two-chunks-per-128-partitions so the per-step
    tanh(c) costs half the ACT columns.  DVE/GPSIMD tensor_tensor requires
    both SBUF inputs at the SAME base partition (NCC_IBIR297), so the gate
    column order inside the weight tiles alternates per chunk parity
    (even chunks [f|i]/[o|g], odd chunks [i|f]/[g|o]); every tensor_tensor
    input pair is then base-aligned and only outputs land cross-base.
  - Cell math: v = sig_i*tanh_g (DVE), u = sig_f*c (GPSIMD), c' = u+v
    (DVE, one full-width op), h = sig_o*tanh(c') written straight into the
    next state tile.
  - Software pipelining: tanh(c) and the h-multiplies for quad k are
    emitted in quad k+1's slot so ACT (the roofline engine here) never
    waits on the quad's own cell chain.
  - Output: final h stays bf16 feature-major [64, 8192]; host transposes
    and casts to f32.
"""

import numpy as np
import ml_dtypes

import concourse.bass as bass
import concourse.mybir as mybir
import concourse.tile as tile_mod
from concourse.tile import TileContext
import bass_rust as _bass_rust
from bass_rust import ScopedClock, VectorClock
from concourse.tile_scheduler import N_PROCS

BF16 = mybir.dt.bfloat16
F32 = mybir.dt.float32
AluOp = mybir.AluOpType
AF = mybir.ActivationFunctionType

T_STEPS = 20
B_FULL = 65536
N_CORES = 8
BC = B_FULL // N_CORES          # 8192
HID = 64
QB = 2048                       # batch per quad (4 psum-bank matmuls)
HQ = QB // 2                    # half-quad (1024)
NQUAD = BC // QB                # 4 quads per step


def _patched_drain_and_barrier(self, tick_clock, wait_clock):
    # This walrus build accepts at most ONE sync-wait per instruction; the
    # stock tail Drain carries one wait per live proc.  Emit one NOP per
    # proc instead, each carrying a single wait.
    gc = tick_clock.global_clock
    for p in range(N_PROCS):
        t = gc[p]
        if t <= 0:
            continue
        nop = self.nc.sync.nop(nofuse=True, hint=f"tail_wait_p{p}")
        wait_clock.add_sem_waits(
            nop.ins,
            ScopedClock(
                {None: VectorClock([t if q == p else 0 for q in range(N_PROCS)])}
            ),
        )
    self.nc.sync.drain()
    self.nc.all_engine_barrier()
    assert self.sems is not None
    popped = self.nc._tile_sem_poison_stack.pop()
    assert popped is self._sem_poison
    self.nc.clear_and_free_semaphores(list(self.sems.allocated().values()))
    self.nc.all_engine_barrier()


tile_mod.TileContext._drain_and_barrier = _patched_drain_and_barrier


def split_excess_waits(nc, max_waits=1):
    """Hoist excess semaphore waits onto same-engine NOPs placed directly
    before the instruction (the engine blocks at the same point)."""
    ctr = 0
    for fn in nc.m.functions:
        for bb in fn.blocks:
            il = bb.instructions
            i = 0
            while i < len(il):
                inst = il[i]
                si = inst.sync_info
                waits = list(si.on_wait) if si is not None and si.on_wait else []
                if len(waits) > max_waits:
                    sem_waits = [w for w in waits if w.sync_type == "semaphore"]
                    other = [w for w in waits if w.sync_type != "semaphore"]
                    keep_n = max(0, max_waits - len(other))
                    keep = other + sem_waits[:keep_n]
                    extra = sem_waits[keep_n:]
                    pos = i
                    for j in range(0, len(extra), max(1, max_waits)):
                        chunk = extra[j:j + max(1, max_waits)]
                        nop = mybir.InstNoOp(name=f"wsplit-{ctr}", ins=[],
                                             outs=[])
                        ctr += 1
                        nop.engine = inst.engine
                        nop.sync_info = _bass_rust.SyncInfo(
                            on_wait=chunk, on_update=[])
                        il.insert(pos, nop)
                        pos += 1
                        i += 1
                    inst.sync_info = _bass_rust.SyncInfo(
                        on_wait=keep,
                        on_update=list(si.on_update) if si.on_update else [])
                i += 1
    return ctr


def host_weights(W_emb, b_emb, W_ih, W_hh, b_ih, b_hh):
    """Four [67, 128] weight blocks: rows 0:64 W_hh.T, row 64 bias,
    rows 65:67 W_x.T; o columns pre-scaled 0.5 (tanh->sigmoid).
    Column order per chunk parity: A-even [f|i], A-odd [i|f],
    B-even [o|g], B-odd [g|o]."""
    W_x = W_ih @ W_emb                      # [256, 2]
    bias = b_ih + b_hh + W_ih @ b_emb       # [256]
    W = np.zeros((67, 256), np.float32)
    W[0:64] = W_hh.T
    W[64] = bias
    W[65:67] = W_x.T
    W[:, 192:256] *= 0.5                    # o gate: tanh(x/2) form
    i_, f_ = W[:, 0:64], W[:, 64:128]
    g_, o_ = W[:, 128:192], W[:, 192:256]
    bf = ml_dtypes.bfloat16
    return (np.concatenate([f_, i_], 1).astype(bf),
            np.concatenate([i_, f_], 1).astype(bf),
            np.concatenate([o_, g_], 1).astype(bf),
            np.concatenate([g_, o_], 1).astype(bf))


def build_nc(split_waits=True, reps=1, loop_reps=1):
    nc = bass.Bass("TRN2", target_bir_lowering=False)
    xt = nc.dram_tensor("xt", [T_STEPS, 2, BC], BF16, kind="ExternalInput")
    wae = nc.dram_tensor("wae", [67, 128], BF16, kind="ExternalInput")
    wao = nc.dram_tensor("wao", [67, 128], BF16, kind="ExternalInput")
    wbe = nc.dram_tensor("wbe", [67, 128], BF16, kind="ExternalInput")
    wbo = nc.dram_tensor("wbo", [67, 128], BF16, kind="ExternalInput")
    hout = nc.dram_tensor("hout", [HID, BC], BF16, kind="ExternalOutput")

    with TileContext(nc) as tc:
        with (
            tc.tile_pool(name="state", bufs=1) as state_pool,
            tc.tile_pool(name="sa", bufs=2) as sa_pool,
            tc.tile_pool(name="tb", bufs=3) as tb_pool,
            tc.tile_pool(name="vv", bufs=2) as v_pool,
            tc.tile_pool(name="uu", bufs=2) as u_pool,
            tc.tile_pool(name="tcp", bufs=2) as tc_pool,
            tc.tile_pool(name="psum", bufs=2, space="PSUM") as psum_pool,
        ):
            WAe = state_pool.tile([67, 128], BF16, tag="WAe")
            WAo = state_pool.tile([67, 128], BF16, tag="WAo")
            WBe = state_pool.tile([67, 128], BF16, tag="WBe")
            WBo = state_pool.tile([67, 128], BF16, tag="WBo")
            for tile, dram in ((WAe, wae), (WAo, wao), (WBe, wbe),
                               (WBo, wbo)):
                nc.sync.dma_start(tile[:], dram[:])

            S0 = state_pool.tile([67, BC], BF16, tag="S0")
            S1 = state_pool.tile([67, BC], BF16, tag="S1")
            Ca = state_pool.tile([128, BC // 2], BF16, tag="Ca")
            Cb = state_pool.tile([128, BC // 2], BF16, tag="Cb")
            S_of = [S0, S1]
            C_of = [Ca, Cb]

            nc.vector.memset(S0[0:64, :], 0.0)
            nc.vector.memset(S0[64:65, :], 1.0)
            nc.vector.memset(S1[64:65, :], 1.0)
            nc.sync.dma_start(S0[65:67, :], xt[0])

            prev = [None]  # [(t, q, TB-tile)] mutable across emit calls

            def emit_step(t):
                S = S_of[t % 2]
                Sn = S_of[(t + 1) % 2]
                Ccur = C_of[t % 2]
                Cprev = C_of[(t - 1) % 2]
                for q in range(NQUAD):
                    if q == 0 and t + 1 < T_STEPS:
                        nc.sync.dma_start(Sn[65:67, :], xt[t + 1])
                    qb = q * QB                 # batch col base
                    qc = q * HQ                 # packed C col base
                    psA = psum_pool.tile([128, QB], F32, tag="ps")
                    psB = psum_pool.tile([128, QB], F32, tag="ps")
                    for j in range(4):
                        rhs = S[0:67, qb + j * 512:qb + (j + 1) * 512]
                        w = WAe if j < 2 else WAo
                        nc.tensor.matmul(psA[:, j * 512:(j + 1) * 512],
                                         lhsT=w[:], rhs=rhs,
                                         start=True, stop=True)
                    for j in range(4):
                        rhs = S[0:67, qb + j * 512:qb + (j + 1) * 512]
                        w = WBe if j < 2 else WBo
                        nc.tensor.matmul(psB[:, j * 512:(j + 1) * 512],
                                         lhsT=w[:], rhs=rhs,
                                         start=True, stop=True)
                    SA = sa_pool.tile([128, QB], BF16, tag="SA")
                    TB = tb_pool.tile([128, QB], BF16, tag="TB")
                    nc.scalar.activation(SA[:], psA[:], AF.Sigmoid)
                    nc.scalar.activation(TB[:], psB[:], AF.Tanh)

                    # ---- tail of previous quad: tanh(c) + h writes ----
                    if prev[0] is not None:
                        emit_tail(*prev[0])

                    # ---- current quad cell math (base-aligned pairs) ----
                    # o fixup in place: sig_o = (tanh+1)*0.5  (DVE 4x)
                    nc.vector.tensor_scalar(
                        TB[0:64, 0:HQ], TB[0:64, 0:HQ], 1.0, 0.5,
                        AluOp.add, AluOp.mult)
                    nc.vector.tensor_scalar(
                        TB[64:128, HQ:QB], TB[64:128, HQ:QB], 1.0, 0.5,
                        AluOp.add, AluOp.mult)
                    V = v_pool.tile([128, HQ], BF16, tag="V")
                    nc.vector.tensor_tensor(
                        V[0:64, :], SA[64:128, 0:HQ], TB[64:128, 0:HQ],
                        AluOp.mult)
                    nc.vector.tensor_tensor(
                        V[64:128, :], SA[0:64, HQ:QB], TB[0:64, HQ:QB],
                        AluOp.mult)
                    if t == 0:
                        nc.vector.tensor_copy(Ccur[:, qc:qc + HQ], V[:])
                    else:
                        U = u_pool.tile([128, HQ], BF16, tag="U")
                        nc.gpsimd.tensor_tensor(
                            U[0:64, :], SA[0:64, 0:HQ],
                            Cprev[0:64, qc:qc + HQ], AluOp.mult)
                        nc.gpsimd.tensor_tensor(
                            U[64:128, :], SA[64:128, HQ:QB],
                            Cprev[64:128, qc:qc + HQ], AluOp.mult)
                        nc.vector.tensor_tensor(
                            Ccur[:, qc:qc + HQ], U[:], V[:], AluOp.add)
                    prev[0] = (t, q, TB)

            def emit_tail(pt, pq, pTB):
                pqc = pq * HQ
                pqb = pq * QB
                pC = C_of[pt % 2]
                pSn = S_of[(pt + 1) % 2]
                TC = tc_pool.tile([128, HQ], BF16, tag="TC")
                nc.scalar.activation(TC[:], pC[:, pqc:pqc + HQ], AF.Tanh)
                nc.vector.tensor_tensor(
                    pSn[0:64, pqb:pqb + HQ],
                    pTB[0:64, 0:HQ], TC[0:64, :], AluOp.mult)
                nc.vector.tensor_tensor(
                    pSn[0:64, pqb + HQ:pqb + QB],
                    pTB[64:128, HQ:QB], TC[64:128, :], AluOp.mult)

            if loop_reps > 1:
                with tc.For_i(0, loop_reps, 1, name="timing_loop"):
                    for tt in range(T_STEPS * reps):
                        emit_step(tt % T_STEPS)
            else:
                for tt in range(T_STEPS * reps):
                    emit_step(tt % T_STEPS)

            # ---- drain the last quad's tail ----
            emit_tail(*prev[0])

            # ---- final h lives in S_of[T%2][0:64, :] -> DRAM ----
            Sfin = S_of[T_STEPS % 2]
            for k in range(8):
                cols = slice(k * (BC // 8), (k + 1) * (BC // 8))
                nc.sync.dma_start(hout[:, cols], Sfin[0:64, cols])
    if split_waits:
        split_excess_waits(nc)
    return nc


_NC_CACHE = {}
LAST_RESULT = None


def kernel(obs_traj, W_emb, b_emb, W_ih, W_hh, b_ih, b_hh):
    global LAST_RESULT
    import os
    from concourse.bass_utils import run_bass_kernel_spmd

    wae, wao, wbe, wbo = host_weights(
        np.asarray(W_emb, dtype=np.float32),
        np.asarray(b_emb, dtype=np.float32),
        np.asarray(W_ih, dtype=np.float32),
        np.asarray(W_hh, dtype=np.float32),
        np.asarray(b_ih, dtype=np.float32),
        np.asarray(b_hh, dtype=np.float32))
    obs = np.asarray(obs_traj)
    in_maps = []
    for c in range(N_CORES):
        sl = obs[:, c * BC:(c + 1) * BC, :]
        xT = np.ascontiguousarray(sl.transpose(0, 2, 1)).astype(
            ml_dtypes.bfloat16)
        in_maps.append({"xt": xT, "wae": wae, "wao": wao,
                        "wbe": wbe, "wbo": wbo})
    if "nc" not in _NC_CACHE:
        _NC_CACHE["nc"] = build_nc()
    res = run_bass_kernel_spmd(
        _NC_CACHE["nc"], in_maps, core_ids=list(range(N_CORES)))
    LAST_RESULT = res
    h = np.concatenate(
        [np.asarray(r["hout"]).T.astype(np.float32) for r in res.results],
        axis=0)
    return h[None]


# revision 3
# speedup vs baseline: 1.2096x; 1.2096x over previous
"""Self-contained Bass/Tile TRN2 kernel v2: 1-layer LSTM encoder, T=20,
batch 65536, hidden 64, data-parallel over batch across 8 NeuronCores.

Feature-major ("orientation B") design — h-state never leaves feature-major
layout, so there is no per-step transpose at all:

  - State tile S[67, 8192] bf16 = [h (rows 0:64); const-1 (row 64);
    x_t (rows 65:67)].  Embedding folded into the input projection on host
    (W_x = W_ih@W_emb, bias = b_ih+b_hh+W_ih@b_emb).
  - Gates per 512-batch chunk via ONE K=67 matmul per gate pair into PSUM:
    pair A = {i,f} (native ACT Sigmoid), pair B = {g,o} (ACT Tanh; o-gate
    weights pre-scaled 0.5 so sigma_o = (tanh+1)/2 via a DVE 4x fixup).
  - The cell state c is packed two-chunks-per-128-partitions so the per-step
    tanh(c) costs half the ACT columns.  DVE/GPSIMD tensor_tensor requires
    both SBUF inputs at the SAME base partition (NCC_IBIR297), so the gate
    column order inside the weight tiles alternates per chunk parity
    (even chunks [f|i]/[o|g], odd chunks [i|f]/[g|o]); every tensor_tensor
    input pair is then base-aligned and only outputs land cross-base.
  - Cell math: v = sig_i*tanh_g (DVE), u = sig_f*c (GPSIMD), c' = u+v
    (DVE, one full-width op), h = sig_o*tanh(c') written straight into the
    next state tile.
  - Software pipelining: tanh(c) and the h-multiplies for quad k are
    emitted in quad k+1's slot so ACT (the roofline engine here) never
    waits on the quad's own cell chain.
  - Output: final h stays bf16 feature-major [64, 8192]; host transposes
    and casts to f32.
"""

import numpy as np
import ml_dtypes

import concourse.bass as bass
import concourse.mybir as mybir
import concourse.tile as tile_mod
from concourse.tile import TileContext
import bass_rust as _bass_rust
from bass_rust import ScopedClock, VectorClock
from concourse.tile_scheduler import N_PROCS

BF16 = mybir.dt.bfloat16
F32 = mybir.dt.float32
AluOp = mybir.AluOpType
AF = mybir.ActivationFunctionType

T_STEPS = 20
B_FULL = 65536
N_CORES = 8
BC = B_FULL // N_CORES          # 8192
HID = 64
QB = 2048                       # batch per quad (4 psum-bank matmuls)
HQ = QB // 2                    # half-quad (1024)
NQUAD = BC // QB                # 4 quads per step


def _patched_drain_and_barrier(self, tick_clock, wait_clock):
    # This walrus build accepts at most ONE sync-wait per instruction; the
    # stock tail Drain carries one wait per live proc.  Emit one NOP per
    # proc instead, each carrying a single wait.
    gc = tick_clock.global_clock
    for p in range(N_PROCS):
        t = gc[p]
        if t <= 0:
            continue
        nop = self.nc.sync.nop(nofuse=True, hint=f"tail_wait_p{p}")
        wait_clock.add_sem_waits(
            nop.ins,
            ScopedClock(
                {None: VectorClock([t if q == p else 0 for q in range(N_PROCS)])}
            ),
        )
    self.nc.sync.drain()
    self.nc.all_engine_barrier()
    assert self.sems is not None
    popped = self.nc._tile_sem_poison_stack.pop()
    assert popped is self._sem_poison
    self.nc.clear_and_free_semaphores(list(self.sems.allocated().values()))
    self.nc.all_engine_barrier()


tile_mod.TileContext._drain_and_barrier = _patched_drain_and_barrier


def split_excess_waits(nc, max_waits=1):
    """Hoist excess semaphore waits onto same-engine NOPs placed directly
    before the instruction (the engine blocks at the same point)."""
    ctr = 0
    for fn in nc.m.functions:
        for bb in fn.blocks:
            il = bb.instructions
            i = 0
            while i < len(il):
                inst = il[i]
                si = inst.sync_info
                waits = list(si.on_wait) if si is not None and si.on_wait else []
                if len(waits) > max_waits:
                    sem_waits = [w for w in waits if w.sync_type == "semaphore"]
                    other = [w for w in waits if w.sync_type != "semaphore"]
                    keep_n = max(0, max_waits - len(other))
                    keep = other + sem_waits[:keep_n]
                    extra = sem_waits[keep_n:]
                    pos = i
                    for j in range(0, len(extra), max(1, max_waits)):
                        chunk = extra[j:j + max(1, max_waits)]
                        nop = mybir.InstNoOp(name=f"wsplit-{ctr}", ins=[],
                                             outs=[])
                        ctr += 1
                        nop.engine = inst.engine
                        nop.sync_info = _bass_rust.SyncInfo(
                            on_wait=chunk, on_update=[])
                        il.insert(pos, nop)
                        pos += 1
                        i += 1
                    inst.sync_info = _bass_rust.SyncInfo(
                        on_wait=keep,
                        on_update=list(si.on_update) if si.on_update else [])
                i += 1
    return ctr


def host_weights(W_emb, b_emb, W_ih, W_hh, b_ih, b_hh):
    """Four [67, 128] weight blocks: rows 0:64 W_hh.T, row 64 bias,
    rows 65:67 W_x.T; o columns pre-scaled 0.5 (tanh->sigmoid).
    Column order per chunk parity: A-even [f|i], A-odd [i|f],
    B-even [o|g], B-odd [g|o]."""
    W_x = W_ih @ W_emb                      # [256, 2]
    bias = b_ih + b_hh + W_ih @ b_emb       # [256]
    W = np.zeros((67, 256), np.float32)
    W[0:64] = W_hh.T
    W[64] = bias
    W[65:67] = W_x.T
    W[:, 192:256] *= 0.5                    # o gate: tanh(x/2) form
    i_, f_ = W[:, 0:64], W[:, 64:128]
    g_, o_ = W[:, 128:192], W[:, 192:256]
    bf = ml_dtypes.bfloat16
    return (np.concatenate([f_, i_], 1).astype(bf),
            np.concatenate([i_, f_], 1).astype(bf),
            np.concatenate([o_, g_], 1).astype(bf),
            np.concatenate([g_, o_], 1).astype(bf))


def build_nc(split_waits=True, reps=1, loop_reps=1, u_on_dve=False,
             deep_bufs=True):
    nc = bass.Bass("TRN2", target_bir_lowering=False)
    xt = nc.dram_tensor("xt", [T_STEPS, 2, BC], BF16, kind="ExternalInput")
    wae = nc.dram_tensor("wae", [67, 128], BF16, kind="ExternalInput")
    wao = nc.dram_tensor("wao", [67, 128], BF16, kind="ExternalInput")
    wbe = nc.dram_tensor("wbe", [67, 128], BF16, kind="ExternalInput")
    wbo = nc.dram_tensor("wbo", [67, 128], BF16, kind="ExternalInput")
    hout = nc.dram_tensor("hout", [HID, BC], BF16, kind="ExternalOutput")

    with TileContext(nc) as tc:
        with (
            tc.tile_pool(name="state", bufs=1) as state_pool,
            tc.tile_pool(name="sa", bufs=3 if deep_bufs else 2) as sa_pool,
            tc.tile_pool(name="tb", bufs=4 if deep_bufs else 3) as tb_pool,
            tc.tile_pool(name="vv", bufs=3 if deep_bufs else 2) as v_pool,
            tc.tile_pool(name="uu", bufs=3 if deep_bufs else 2) as u_pool,
            tc.tile_pool(name="tcp", bufs=3 if deep_bufs else 2) as tc_pool,
            tc.tile_pool(name="psum", bufs=2, space="PSUM") as psum_pool,
        ):
            WAe = state_pool.tile([67, 128], BF16, tag="WAe")
            WAo = state_pool.tile([67, 128], BF16, tag="WAo")
            WBe = state_pool.tile([67, 128], BF16, tag="WBe")
            WBo = state_pool.tile([67, 128], BF16, tag="WBo")
            for tile, dram in ((WAe, wae), (WAo, wao), (WBe, wbe),
                               (WBo, wbo)):
                nc.sync.dma_start(tile[:], dram[:])

            S0 = state_pool.tile([67, BC], BF16, tag="S0")
            S1 = state_pool.tile([67, BC], BF16, tag="S1")
            Ca = state_pool.tile([128, BC // 2], BF16, tag="Ca")
            Cb = state_pool.tile([128, BC // 2], BF16, tag="Cb")
            S_of = [S0, S1]
            C_of = [Ca, Cb]

            nc.vector.memset(S0[0:64, :], 0.0)
            nc.vector.memset(S0[64:65, :], 1.0)
            nc.vector.memset(S1[64:65, :], 1.0)
            nc.sync.dma_start(S0[65:67, :], xt[0])

            prev = [None]  # [(t, q, TB-tile)] mutable across emit calls

            def emit_step(t):
                S = S_of[t % 2]
                Sn = S_of[(t + 1) % 2]
                Ccur = C_of[t % 2]
                Cprev = C_of[(t - 1) % 2]
                for q in range(NQUAD):
                    if q == 0 and t + 1 < T_STEPS:
                        nc.sync.dma_start(Sn[65:67, :], xt[t + 1])
                    qb = q * QB                 # batch col base
                    qc = q * HQ                 # packed C col base
                    psA = psum_pool.tile([128, QB], F32, tag="ps")
                    psB = psum_pool.tile([128, QB], F32, tag="ps")
                    for j in range(4):
                        rhs = S[0:67, qb + j * 512:qb + (j + 1) * 512]
                        w = WAe if j < 2 else WAo
                        nc.tensor.matmul(psA[:, j * 512:(j + 1) * 512],
                                         lhsT=w[:], rhs=rhs,
                                         start=True, stop=True)
                    for j in range(4):
                        rhs = S[0:67, qb + j * 512:qb + (j + 1) * 512]
                        w = WBe if j < 2 else WBo
                        nc.tensor.matmul(psB[:, j * 512:(j + 1) * 512],
                                         lhsT=w[:], rhs=rhs,
                                         start=True, stop=True)
                    SA = sa_pool.tile([128, QB], BF16, tag="SA")
                    TB = tb_pool.tile([128, QB], BF16, tag="TB")
                    nc.scalar.activation(SA[:], psA[:], AF.Sigmoid)
                    nc.scalar.activation(TB[:], psB[:], AF.Tanh)

                    # ---- tail of previous quad: tanh(c) + h writes ----
                    if prev[0] is not None:
                        emit_tail(*prev[0])

                    # ---- current quad cell math (base-aligned pairs) ----
                    # o fixup in place: sig_o = (tanh+1)*0.5  (DVE 4x)
                    nc.vector.tensor_scalar(
                        TB[0:64, 0:HQ], TB[0:64, 0:HQ], 1.0, 0.5,
                        AluOp.add, AluOp.mult)
                    nc.vector.tensor_scalar(
                        TB[64:128, HQ:QB], TB[64:128, HQ:QB], 1.0, 0.5,
                        AluOp.add, AluOp.mult)
                    V = v_pool.tile([128, HQ], BF16, tag="V")
                    nc.vector.tensor_tensor(
                        V[0:64, :], SA[64:128, 0:HQ], TB[64:128, 0:HQ],
                        AluOp.mult)
                    nc.vector.tensor_tensor(
                        V[64:128, :], SA[0:64, HQ:QB], TB[0:64, HQ:QB],
                        AluOp.mult)
                    if t == 0:
                        nc.vector.tensor_copy(Ccur[:, qc:qc + HQ], V[:])
                    else:
                        U = u_pool.tile([128, HQ], BF16, tag="U")
                        u_eng = nc.vector if u_on_dve else nc.gpsimd
                        u_eng.tensor_tensor(
                            U[0:64, :], SA[0:64, 0:HQ],
                            Cprev[0:64, qc:qc + HQ], AluOp.mult)
                        u_eng.tensor_tensor(
                            U[64:128, :], SA[64:128, HQ:QB],
                            Cprev[64:128, qc:qc + HQ], AluOp.mult)
                        nc.vector.tensor_tensor(
                            Ccur[:, qc:qc + HQ], U[:], V[:], AluOp.add)
                    prev[0] = (t, q, TB)

            def emit_tail(pt, pq, pTB):
                pqc = pq * HQ
                pqb = pq * QB
                pC = C_of[pt % 2]
                pSn = S_of[(pt + 1) % 2]
                TC = tc_pool.tile([128, HQ], BF16, tag="TC")
                nc.scalar.activation(TC[:], pC[:, pqc:pqc + HQ], AF.Tanh)
                nc.vector.tensor_tensor(
                    pSn[0:64, pqb:pqb + HQ],
                    pTB[0:64, 0:HQ], TC[0:64, :], AluOp.mult)
                nc.vector.tensor_tensor(
                    pSn[0:64, pqb + HQ:pqb + QB],
                    pTB[64:128, HQ:QB], TC[64:128, :], AluOp.mult)

            if loop_reps > 1:
                with tc.For_i(0, loop_reps, 1, name="timing_loop"):
                    for tt in range(T_STEPS * reps):
                        emit_step(tt % T_STEPS)
            else:
                for tt in range(T_STEPS * reps):
                    emit_step(tt % T_STEPS)

            # ---- drain the last quad's tail ----
            emit_tail(*prev[0])

            # ---- final h lives in S_of[T%2][0:64, :] -> DRAM ----
            Sfin = S_of[T_STEPS % 2]
            for k in range(8):
                cols = slice(k * (BC // 8), (k + 1) * (BC // 8))
                nc.sync.dma_start(hout[:, cols], Sfin[0:64, cols])
    if split_waits:
        split_excess_waits(nc)
    return nc


_NC_CACHE = {}
LAST_RESULT = None


def kernel(obs_traj, W_emb, b_emb, W_ih, W_hh, b_ih, b_hh):
    global LAST_RESULT
    import os
    from concourse.bass_utils import run_bass_kernel_spmd

    wae, wao, wbe, wbo = host_weights(
        np.asarray(W_emb, dtype=np.float32),
        np.asarray(b_emb, dtype=np.float32),
        np.asarray(W_ih, dtype=np.float32),
        np.asarray(W_hh, dtype=np.float32),
        np.asarray(b_ih, dtype=np.float32),
        np.asarray(b_hh, dtype=np.float32))
    obs = np.asarray(obs_traj)
    in_maps = []
    for c in range(N_CORES):
        sl = obs[:, c * BC:(c + 1) * BC, :]
        xT = np.ascontiguousarray(sl.transpose(0, 2, 1)).astype(
            ml_dtypes.bfloat16)
        in_maps.append({"xt": xT, "wae": wae, "wao": wao,
                        "wbe": wbe, "wbo": wbo})
    if "nc" not in _NC_CACHE:
        _NC_CACHE["nc"] = build_nc()
    res = run_bass_kernel_spmd(
        _NC_CACHE["nc"], in_maps, core_ids=list(range(N_CORES)))
    LAST_RESULT = res
    h = np.concatenate(
        [np.asarray(r["hout"]).T.astype(np.float32) for r in res.results],
        axis=0)
    return h[None]
